# revision 1
# baseline (speedup 1.0000x reference)
"""Half-Chamfer distance kernel for Trainium2 (8 NeuronCores).

Problem: prediction [4, 8192, 3], ground_truth [4, 8192, 3] (f32).
out[b] = mean_n min_m ||pred[b,n] - gt[b,m]||^2

Sharding: core c -> (batch b = c//2, N-half h = c%2). Each core computes
min over all M=8192 gt points for its 4096 prediction points, clamps,
row-sums; host combines the per-core [128] partial sums.

Device algorithm (per core):
  d2[n,m] = x2[n] + y2[m] - 2 x.y[m]  as K=5 matmuls (float32r):
    stationary rows [x0, x1, x2, x2n, 1] (pred points on columns)
    moving rows     [-2 y0, -2 y1, -2 y2, 1, y2]
  Even-m columns (E) and odd-m columns (O) form separate moving tensors,
  so min(E[n,j], O[n,j]) = min over the m-pair j.  Per chunk:
    PE     -> E, O into PSUM                       (4 matmuls, FD=512)
    ScalarE-> copy O PSUM->SBUF                    (1 elem/cycle)
    VectorE-> u = min(E_psum, O_sbuf)  [TT, 1x]
              chunkmin = reduce_min(u) [SBUF 2x mode]
  Chunk minima collect per n-tile, a final reduce gives d_x; relu-clamp
  and row-sum on device, host sums 128 partials per core.
"""

import numpy as np

import concourse.bass as bass
import concourse.mybir as mybir
from concourse.bass_utils import run_bass_kernel_spmd
from concourse.tile import TileContext

B = 4
N = 8192
M = 8192
D = 3
N_CORES = 8
N_SH = N // 2          # 4096 prediction points per core
J = M // 2             # 4096 m-pairs
JC = 512               # pair-chunk per matmul (1 PSUM bank)
NTILES = N_SH // 128   # 32 n-tiles of 128 partitions
CHUNKS = J // JC       # 8 matmul chunks per n-tile
CPAIRS = CHUNKS // 2   # 4 TT+reduce groups ([128, 1024]) per n-tile

F32 = mybir.dt.float32
F32R = mybir.dt.float32r

_CACHED_NC = None


def _build_nc(mm_dtype=F32R):
    nc = bass.Bass()
    statx_d = nc.declare_dram_parameter("statx", [5, N_SH], F32, isOutput=False)
    emov_d = nc.declare_dram_parameter("emov", [5, J], F32, isOutput=False)
    omov_d = nc.declare_dram_parameter("omov", [5, J], F32, isOutput=False)
    out_d = nc.declare_dram_parameter("out", [128, 1], F32, isOutput=True)

    with TileContext(nc) as tc:
        with (
            tc.tile_pool(name="const", bufs=1) as cpool,
            tc.tile_pool(name="osb", bufs=3) as opool,
            tc.tile_pool(name="u", bufs=3) as upool,
            tc.tile_pool(name="cm", bufs=2) as cmpool,
            tc.tile_pool(name="ps_e", bufs=2, space="PSUM") as epool,
            tc.tile_pool(name="ps_o", bufs=2, space="PSUM") as gpool,
        ):
            statx_f = cpool.tile([5, N_SH], F32, tag="statx_f")
            emov_f = cpool.tile([5, J], F32, tag="emov_f")
            omov_f = cpool.tile([5, J], F32, tag="omov_f")
            dx_all = cpool.tile([128, NTILES], F32, tag="dx")
            nc.sync.dma_start(out=statx_f[:], in_=statx_d[:])
            nc.sync.dma_start(out=emov_f[:], in_=emov_d[:])
            nc.sync.dma_start(out=omov_f[:], in_=omov_d[:])

            # fp32r operands must be produced by a rounding op (BIR rule)
            statx = cpool.tile([5, N_SH], mm_dtype, tag="statx")
            emov = cpool.tile([5, J], mm_dtype, tag="emov")
            omov = cpool.tile([5, J], mm_dtype, tag="omov")
            nc.vector.tensor_copy(out=emov[:], in_=emov_f[:])
            nc.vector.tensor_copy(out=omov[:], in_=omov_f[:])
            nc.vector.tensor_copy(out=statx[:], in_=statx_f[:])

            for t in range(NTILES):
                lhs = statx[:, t * 128:(t + 1) * 128]
                cmins = cmpool.tile([128, CPAIRS], F32, tag="cmins")
                for cp in range(CPAIRS):
                    e2 = epool.tile([128, 2 * JC], F32, tag="e2")
                    for k in range(2):
                        c = 2 * cp + k
                        nc.tensor.matmul(
                            out=e2[:, k * JC:(k + 1) * JC],
                            lhsT=lhs,
                            rhs=emov[:, c * JC:(c + 1) * JC],
                            start=True, stop=True,
                        )
                    o2 = gpool.tile([128, 2 * JC], F32, tag="o2")
                    for k in range(2):
                        c = 2 * cp + k
                        nc.tensor.matmul(
                            out=o2[:, k * JC:(k + 1) * JC],
                            lhsT=lhs,
                            rhs=omov[:, c * JC:(c + 1) * JC],
                            start=True, stop=True,
                        )
                    osb = opool.tile([128, 2 * JC], F32, tag="osb")
                    nc.scalar.copy(out=osb[:], in_=o2[:])
                    u = upool.tile([128, 2 * JC], F32, tag="u")
                    nc.vector.tensor_tensor(
                        out=u[:], in0=e2[:], in1=osb[:],
                        op=mybir.AluOpType.min,
                    )
                    nc.vector.tensor_reduce(
                        out=cmins[:, cp:cp + 1], in_=u[:],
                        axis=mybir.AxisListType.X, op=mybir.AluOpType.min,
                    )
                nc.vector.tensor_reduce(
                    out=dx_all[:, t:t + 1], in_=cmins[:],
                    axis=mybir.AxisListType.X, op=mybir.AluOpType.min,
                )

            # clamp at 0 (matches reference's maximum(d2, 0) before min)
            nc.vector.tensor_scalar_max(
                out=dx_all[:], in0=dx_all[:], scalar1=0.0
            )
            dxsum = cpool.tile([128, 1], F32, tag="dxsum")
            nc.vector.tensor_reduce(
                out=dxsum[:], in_=dx_all[:],
                axis=mybir.AxisListType.X, op=mybir.AluOpType.add,
            )
            nc.sync.dma_start(out=out_d[:], in_=dxsum[:])

    _legalize_for_walrus(nc)
    return nc


def _legalize_for_walrus(nc, max_waits=1):
    """This container's walrus encodes at most one sync-wait per
    instruction (fused-LW matmuls, drains, ...) and cannot encode
    EVENT_SEMAPHORE_RANGE_CLEAR at all.  Spill extra waits onto
    standalone NoOps queued just before on the same engine, and drop the
    tail sem range-clear."""
    RANGE_CLEAR_OPCODE = 176
    for f in nc.m.functions:
        for blk in f.blocks:
            out = []
            for inst in blk.instructions:
                if (
                    type(inst).__name__ == "InstISA"
                    and getattr(inst, "isa_opcode", None) == RANGE_CLEAR_OPCODE
                ):
                    continue
                si = inst.sync_info
                if si is not None and len(si.on_wait) > max_waits:
                    waits = list(si.on_wait)
                    for w in waits[:-max_waits]:
                        out.append(mybir.InstNoOp(
                            name=nc.get_next_instruction_name(),
                            engine=inst.engine,
                            sync_info=mybir.SyncInfo(
                                on_wait=[w], on_update=[]),
                        ))
                    inst.sync_info = mybir.SyncInfo(
                        on_wait=waits[-max_waits:],
                        on_update=list(si.on_update),
                    )
                out.append(inst)
            blk.instructions = out


def _get_nc():
    global _CACHED_NC
    if _CACHED_NC is None:
        _CACHED_NC = _build_nc()
    return _CACHED_NC


def _prep_core_inputs(x, y):
    """x: [N_SH, 3] f32 pred slice; y: [M, 3] f32 gt batch. f64 math."""
    x = x.astype(np.float64)
    y = y.astype(np.float64)
    x2 = (x * x).sum(-1)
    ones = np.ones_like(x2)
    statx = np.stack([x[:, 0], x[:, 1], x[:, 2], x2, ones])  # [5, N_SH]

    y2 = (y * y).sum(-1)
    mov = np.stack([
        -2.0 * y[:, 0], -2.0 * y[:, 1], -2.0 * y[:, 2],
        np.ones(M), y2,
    ])                                                        # [5, M]
    emov = mov[:, 0::2]
    omov = mov[:, 1::2]
    return {
        "statx": np.ascontiguousarray(statx, dtype=np.float32),
        "emov": np.ascontiguousarray(emov, dtype=np.float32),
        "omov": np.ascontiguousarray(omov, dtype=np.float32),
    }


def kernel(prediction, ground_truth, _trace=False, _trace_kwargs=None):
    prediction = np.asarray(prediction, dtype=np.float32)
    ground_truth = np.asarray(ground_truth, dtype=np.float32)
    assert prediction.shape == (B, N, D)
    assert ground_truth.shape == (B, M, D)

    nc = _get_nc()
    in_maps = []
    for c in range(N_CORES):
        b, h = c // 2, c % 2
        x = prediction[b, h * N_SH:(h + 1) * N_SH]
        in_maps.append(_prep_core_inputs(x, ground_truth[b]))

    kw = {}
    if _trace:
        kw = {"trace": True, "trace_cores": [0]}
        if _trace_kwargs:
            kw.update(_trace_kwargs)
    res = run_bass_kernel_spmd(nc, in_maps, list(range(N_CORES)), **kw)

    out = np.zeros(B, dtype=np.float64)
    for c in range(N_CORES):
        out[c // 2] += res.results[c]["out"].astype(np.float64).sum()
    out = (out / N).astype(np.float32)
    if _trace:
        kernel.last_result = res
    return out

